# revision 2
# baseline (speedup 1.0000x reference)
"""Trainium2 Bass kernel for GAT-style attention softmax (CochainMessagePassing).

Computes, for inputs
    x       [4, 4, 1024, 512]  f32
    attn_w  [4, 4, 8, 1024, 128] f32
the output
    out     [4, 4, 1024, 8, 1024] f32
where per (b, n, head h):
    xh   = x[b, n, :, h*64:(h+1)*64]            # [1024, 64]
    a2   = attn_w[b, n, h, :, 64:128]           # [1024, 64]
    e    = a2 @ xh.T                            # [1024, 1024]
    out[b, n, i, h, j] = softmax_j(e_self[i] + e[i, j]) = softmax_j(e[i, j])
(e_self is constant along the softmax axis so it cancels; a1 is never needed).

Sharding: the 16 (b, n) slabs are split 2-per-core across 8 NeuronCores
(pure data parallel, no collectives).
"""

import sys

sys.path.insert(0, "/opt/trn_rl_repo")

from contextlib import ExitStack

import numpy as np

import concourse.bass as bass
import concourse.tile as tile
from concourse import mybir
from concourse.bass_utils import run_bass_kernel_spmd
from concourse.masks import make_identity

NUM_CORES = 8
SLABS_PER_CORE = 2  # (b, n) pairs per core
N_C = 1024  # complexes
D = 512
H = 8  # heads
DH = 64  # head dim
NIB = N_C // 128  # i-blocks per slab

F32 = mybir.dt.float32
F32R = mybir.dt.float32r

# score matmuls in float32r (full 4-byte operands, 1 cycle/row for N>=256)
USE_F32R = False


def make_pools(ctx: ExitStack, tc: tile.TileContext):
    nc = tc.nc
    pools = {}
    pools["const"] = ctx.enter_context(tc.tile_pool(name="const", bufs=1))
    pools["xstage"] = ctx.enter_context(tc.tile_pool(name="xstage", bufs=4))
    pools["xT"] = ctx.enter_context(tc.tile_pool(name="xT", bufs=2))
    pools["a2stage"] = ctx.enter_context(tc.tile_pool(name="a2stage", bufs=2))
    pools["a2T"] = ctx.enter_context(tc.tile_pool(name="a2T", bufs=2))
    pools["exp"] = ctx.enter_context(tc.tile_pool(name="exp", bufs=4))
    pools["outp"] = ctx.enter_context(tc.tile_pool(name="outp", bufs=4))
    pools["stat"] = ctx.enter_context(tc.tile_pool(name="stat", bufs=8))
    pools["tpsum"] = ctx.enter_context(tc.tile_pool(name="tpsum", bufs=2, space="PSUM"))
    pools["spsum"] = ctx.enter_context(tc.tile_pool(name="spsum", bufs=3, space="PSUM"))
    identity = pools["const"].tile([128, 128], F32)
    make_identity(nc, identity[:])
    pools["identity"] = identity
    return pools


def build_kernel_body(pools, tc: tile.TileContext, out_ap, x_ap, w_ap):
    nc = tc.nc
    xstage = pools["xstage"]
    xT_pool = pools["xT"]
    a2stage = pools["a2stage"]
    a2T_pool = pools["a2T"]
    exp_pool = pools["exp"]
    outp = pools["outp"]
    stat_pool = pools["stat"]
    tpsum = pools["tpsum"]
    spsum = pools["spsum"]
    identity = pools["identity"]

    mm_dt = F32R if USE_F32R else F32

    for s in range(SLABS_PER_CORE):
        # ---- transpose x[s]: [1024 (j), 512 (d)] -> xT [512 (d), 1024 (j)] ----
        # xT packed as one [128, 4096] tile: xT[dd, p*1024 + j] = x[s, j, p*128+dd]
        xT = xT_pool.tile([128, 4 * N_C], mm_dt)
        for jb in range(NIB):
            x_sb = xstage.tile([128, D], F32)
            nc.gpsimd.dma_start(x_sb[:], x_ap[s, jb * 128 : (jb + 1) * 128, :])
            for p in range(4):
                ps = tpsum.tile([128, 128], F32)
                nc.tensor.transpose(ps[:], x_sb[:, p * 128 : (p + 1) * 128], identity[:])
                nc.any.tensor_copy(
                    out=xT[:, p * N_C + jb * 128 : p * N_C + (jb + 1) * 128],
                    in_=ps[:],
                )

        # ---- per head-pair q: heads (2q, 2q+1) ----
        for q in range(4):
            # load a2 for both heads, interleaved per 128-col block:
            # a2s[:, ib*128 + hh*64 + k] = attn_w[s, 2q+hh, ib*128 + i, 64 + k]
            a2s = a2stage.tile([128, N_C], F32)
            a2s_r = a2s[:].rearrange("p (a c) -> p a c", c=128)
            for hh in range(2):
                h = 2 * q + hh
                src = w_ap[s, h, :, DH : 2 * DH].rearrange("(a p) k -> p a k", p=128)
                nc.gpsimd.dma_start(a2s_r[:, :, hh * DH : (hh + 1) * DH], src)

            # transpose to a2T [128 (k of pair), 1024 (i)]:
            # a2T[hh*64 + k, i] = a2 of head (2q+hh) at [i, k]
            a2T = a2T_pool.tile([128, N_C], mm_dt)
            for ib in range(NIB):
                ps = tpsum.tile([128, 128], F32)
                nc.tensor.transpose(ps[:], a2s[:, ib * 128 : (ib + 1) * 128], identity[:])
                nc.any.tensor_copy(out=a2T[:, ib * 128 : (ib + 1) * 128], in_=ps[:])

            # ---- scores + softmax per head, per i-block ----
            for hh in range(2):
                h = 2 * q + hh
                # rhs: xT rows h*64..h*64+64 = partition offset hh*64 of block p=q
                rhs_all = xT[hh * DH : (hh + 1) * DH, q * N_C : (q + 1) * N_C]
                for ib in range(NIB):
                    lhsT = a2T[hh * DH : (hh + 1) * DH, ib * 128 : (ib + 1) * 128]
                    psc = spsum.tile([128, N_C], F32)
                    for jc in range(2):
                        nc.tensor.matmul(
                            psc[:, jc * 512 : (jc + 1) * 512],
                            lhsT,
                            rhs_all[:, jc * 512 : (jc + 1) * 512],
                            start=True,
                            stop=True,
                        )
                    expt = exp_pool.tile([128, N_C], F32)
                    sums = stat_pool.tile([128, 1], F32, tag="sums")
                    nc.scalar.activation(
                        expt[:],
                        psc[:],
                        mybir.ActivationFunctionType.Exp,
                        accum_out=sums[:],
                    )
                    rec = stat_pool.tile([128, 1], F32, tag="rec")
                    nc.vector.reciprocal(rec[:], sums[:])
                    outt = outp.tile([128, N_C], F32)
                    nc.vector.tensor_scalar_mul(outt[:], expt[:], rec[:])
                    nc.sync.dma_start(
                        out_ap[s, ib * 128 : (ib + 1) * 128, h, :], outt[:]
                    )


def _split_multi_waits(nc):
    """walrus's per-instruction codegen structs hold only one embedded sync
    wait; hoist multi-wait instructions' waits onto standalone same-engine
    wait instructions placed immediately before them (program order on the
    sequencer preserves semantics)."""
    ctr = 0
    for f in nc.m.functions:
        for blk in f.blocks:
            out = []
            changed = False
            for inst in blk.instructions:
                tname = type(inst).__name__
                si = inst.sync_info
                if (
                    tname != "InstEventSemaphore"
                    and si is not None
                    and si.on_wait
                    and len(si.on_wait) > 1
                ):
                    for w in si.on_wait:
                        wi = mybir.InstEventSemaphore(name=f"WSPLIT-{ctr}")
                        ctr += 1
                        wi.engine = inst.engine
                        wi.sync_info = mybir.SyncInfo(on_wait=[w], on_update=[])
                        out.append(wi)
                    inst.sync_info = mybir.SyncInfo(
                        on_wait=[], on_update=list(si.on_update)
                    )
                    changed = True
                out.append(inst)
            if changed:
                blk.instructions = out
    return ctr


def build_bass(bench_repeats=None, split_waits=True):
    nc = bass.Bass("TRN2", target_bir_lowering=False, debug=False)
    if bench_repeats is None:
        x_ap = nc.dram_tensor(
            "x", [SLABS_PER_CORE, N_C, D], F32, kind="ExternalInput"
        ).ap()
        w_ap = nc.dram_tensor(
            "attn_w", [SLABS_PER_CORE, H, N_C, 2 * DH], F32, kind="ExternalInput"
        ).ap()
        out_ap = nc.dram_tensor(
            "out", [SLABS_PER_CORE, N_C, H, N_C], F32, kind="ExternalOutput"
        ).ap()
        with tile.TileContext(nc) as tc:
            with ExitStack() as ctx:
                pools = make_pools(ctx, tc)
                build_kernel_body(pools, tc, out_ap, x_ap, w_ap)
    else:
        # bench variant: all big tensors are device-internal (no host I/O);
        # tiny external in/out keep the custom-call ABI happy. Internal
        # inputs are zeroed once, then the body runs `bench_repeats` times
        # (unrolled; For_i trips a walrus InstISA codegen bug).
        x_ap = nc.dram_tensor("xi", [SLABS_PER_CORE, N_C, D], F32).ap()
        w_ap = nc.dram_tensor("wi", [SLABS_PER_CORE, H, N_C, 2 * DH], F32).ap()
        out_ap = nc.dram_tensor("oi", [SLABS_PER_CORE, N_C, H, N_C], F32).ap()
        tin = nc.dram_tensor("tin", [1, 4], F32, kind="ExternalInput").ap()
        tout = nc.dram_tensor("tout", [1, 4], F32, kind="ExternalOutput").ap()
        with tile.TileContext(nc) as tc:
            with ExitStack() as ctx:
                pools = make_pools(ctx, tc)
                tiny = pools["const"].tile([1, 4], F32)
                nc.gpsimd.dma_start(tiny[:], tin[:, :])
                nc.gpsimd.dma_start(tout[:, :], tiny[:])
                zt = pools["const"].tile([128, 4 * N_C], F32)
                nc.vector.memset(zt[:], 0.0)
                x_flat = x_ap.rearrange("s (a p) d -> (s a) p d", p=128)
                for t in range(x_flat.shape[0]):
                    nc.gpsimd.dma_start(x_flat[t], zt[:, :D])
                w_flat = w_ap.rearrange("s h (a p) k -> (s h a) p k", p=128)
                for t in range(w_flat.shape[0]):
                    nc.gpsimd.dma_start(w_flat[t], zt[:, : 2 * DH])
                for _ in range(bench_repeats):
                    build_kernel_body(pools, tc, out_ap, x_ap, w_ap)
    if split_waits:
        _split_multi_waits(nc)
    return nc


_NC_CACHE = None


def _get_nc():
    global _NC_CACHE
    if _NC_CACHE is None:
        _NC_CACHE = build_bass()
    return _NC_CACHE


def kernel(x: np.ndarray, attn_w: np.ndarray, _trace: bool = False):
    assert x.shape == (4, 4, N_C, D), x.shape
    assert attn_w.shape == (4, 4, H, N_C, 2 * DH), attn_w.shape
    xs = np.ascontiguousarray(x, dtype=np.float32).reshape(16, N_C, D)
    ws = np.ascontiguousarray(attn_w, dtype=np.float32).reshape(16, H, N_C, 2 * DH)
    in_maps = [
        {
            "x": np.ascontiguousarray(xs[2 * c : 2 * c + 2]),
            "attn_w": np.ascontiguousarray(ws[2 * c : 2 * c + 2]),
        }
        for c in range(NUM_CORES)
    ]
    nc = _get_nc()
    res = run_bass_kernel_spmd(
        nc, in_maps, core_ids=list(range(NUM_CORES)), trace=_trace
    )
    out = np.concatenate([res.results[c]["out"] for c in range(NUM_CORES)], axis=0)
    if _trace:
        kernel.last_exec_time_ns = res.exec_time_ns
        it = res.instructions_and_trace
        kernel.last_trace_path = it[1] if it else None
    return out.reshape(4, 4, N_C, H, N_C)


kernel.last_exec_time_ns = None
kernel.last_trace_path = None



# revision 3
# speedup vs baseline: 1.7047x; 1.7047x over previous
"""Trainium2 Bass kernel for GAT-style attention softmax (CochainMessagePassing).

Computes, for inputs
    x       [4, 4, 1024, 512]  f32
    attn_w  [4, 4, 8, 1024, 128] f32
the output
    out     [4, 4, 1024, 8, 1024] f32
where per (b, n, head h):
    xh   = x[b, n, :, h*64:(h+1)*64]            # [1024, 64]
    a2   = attn_w[b, n, h, :, 64:128]           # [1024, 64]
    e    = a2 @ xh.T                            # [1024, 1024]
    out[b, n, i, h, j] = softmax_j(e_self[i] + e[i, j]) = softmax_j(e[i, j])
(e_self is constant along the softmax axis so it cancels; a1 is never needed).

Sharding: the 16 (b, n) slabs are split 2-per-core across 8 NeuronCores
(pure data parallel, no collectives).

v2 pipeline per slab:
  - DMA x -> SBUF, 32 PE transposes -> xT [128, 4x1024] f32
  - DMA a2 (head-pair interleaved) -> SBUF, 32 PE transposes -> a2T [128, 4x1024]
  - per i-block (8) x head (8): two f32r matmuls (N=512) -> PSUM scores,
    ACT exp -> bf16 in out_sb slice + accum_out -> stats column,
    DVE reciprocal (batched [128,8]) + in-place tensor_scalar mult (4x mode)
  - one 2 MB DMA per (slab, i-block): out_sb [128, 8*1024] bf16 -> HBM
Output is stored bf16 on device and upcast to f32 on the host (values are
softmax probabilities; bf16 keeps rel err ~4e-3, well under tolerance).
"""

import sys

sys.path.insert(0, "/opt/trn_rl_repo")

from contextlib import ExitStack

import numpy as np

import concourse.bass as bass
import concourse.tile as tile
from concourse import mybir
from concourse.bass_utils import run_bass_kernel_spmd
from concourse.masks import make_identity

NUM_CORES = 8
SLABS_PER_CORE = 2  # (b, n) pairs per core
N_C = 1024  # complexes
D = 512
H = 8  # heads
DH = 64  # head dim
NIB = N_C // 128  # i-blocks per slab

F32 = mybir.dt.float32
F32R = mybir.dt.float32r
BF16 = mybir.dt.bfloat16

# score matmuls in float32r (full 4-byte operands, 1 cycle/row for N>=256)
USE_F32R = True
OUT_DT = BF16


def make_pools(ctx: ExitStack, tc: tile.TileContext):
    nc = tc.nc
    pools = {}
    pools["const"] = ctx.enter_context(tc.tile_pool(name="const", bufs=1))
    pools["xstage"] = ctx.enter_context(tc.tile_pool(name="xstage", bufs=2))
    pools["xT"] = ctx.enter_context(tc.tile_pool(name="xT", bufs=2))
    pools["a2stage"] = ctx.enter_context(tc.tile_pool(name="a2stage", bufs=2))
    pools["a2T"] = ctx.enter_context(tc.tile_pool(name="a2T", bufs=2))
    pools["outp"] = ctx.enter_context(tc.tile_pool(name="outp", bufs=3))
    pools["stat"] = ctx.enter_context(tc.tile_pool(name="stat", bufs=8))
    pools["tpsum"] = ctx.enter_context(tc.tile_pool(name="tpsum", bufs=2, space="PSUM"))
    pools["spsum"] = ctx.enter_context(tc.tile_pool(name="spsum", bufs=3, space="PSUM"))
    identity = pools["const"].tile([128, 128], F32)
    make_identity(nc, identity[:])
    pools["identity"] = identity
    return pools


def build_kernel_body(pools, tc: tile.TileContext, out_ap, x_ap, w_ap):
    nc = tc.nc
    xstage = pools["xstage"]
    xT_pool = pools["xT"]
    a2stage = pools["a2stage"]
    a2T_pool = pools["a2T"]
    outp = pools["outp"]
    stat_pool = pools["stat"]
    tpsum = pools["tpsum"]
    spsum = pools["spsum"]
    identity = pools["identity"]

    mm_dt = F32R if USE_F32R else F32

    for s in range(SLABS_PER_CORE):
        # ---- stage x[s]: one DMA into [128, 8*512] (jb-blocked) ----
        # x_sb[p, jb*512 + d] = x[s, jb*128 + p, d]
        x_sb = xstage.tile([128, NIB * D], F32)
        nc.sync.dma_start(
            x_sb[:].rearrange("p (a d) -> p a d", a=NIB),
            x_ap[s].rearrange("(a p) d -> p a d", p=128),
        )

        # ---- transpose x[s]: [1024 (j), 512 (d)] -> xT [512 (d), 1024 (j)] ----
        # xT packed as one [128, 4096] tile: xT[dd, p*1024 + j] = x[s, j, p*128+dd]
        xT = xT_pool.tile([128, 4 * N_C], mm_dt)
        for jb in range(NIB):
            for p in range(4):
                ps = tpsum.tile([128, 128], F32)
                nc.tensor.transpose(
                    ps[:], x_sb[:, jb * D + p * 128 : jb * D + (p + 1) * 128], identity[:]
                )
                nc.vector.tensor_copy(
                    out=xT[:, p * N_C + jb * 128 : p * N_C + (jb + 1) * 128],
                    in_=ps[:],
                )

        # ---- stage + transpose a2 for all 4 head-pairs q ----
        # a2s[q]: [128, N_C] with a2s[p, ib*128 + hh*64 + k] = w[s, 2q+hh, ib*128+p, 64+k]
        # a2T packed as one [128, 4*N_C]: a2T[hh*64+k, q*N_C + i] = a2 of head 2q+hh at [i, k]
        a2s = a2stage.tile([128, 4 * N_C], F32)
        a2T = a2T_pool.tile([128, 4 * N_C], mm_dt)
        for q in range(4):
            a2s_q = a2s[:, q * N_C : (q + 1) * N_C]
            a2s_r = a2s_q.rearrange("p (a c) -> p a c", c=128)
            for hh in range(2):
                h = 2 * q + hh
                src = w_ap[s, h, :, DH : 2 * DH].rearrange("(a p) k -> p a k", p=128)
                nc.gpsimd.dma_start(a2s_r[:, :, hh * DH : (hh + 1) * DH], src)
            for ib in range(NIB):
                ps = tpsum.tile([128, 128], F32)
                nc.tensor.transpose(
                    ps[:], a2s_q[:, ib * 128 : (ib + 1) * 128], identity[:]
                )
                nc.vector.tensor_copy(
                    out=a2T[:, q * N_C + ib * 128 : q * N_C + (ib + 1) * 128],
                    in_=ps[:],
                )

        # ---- scores + softmax, i-block major; one output DMA per i-block ----
        for ib in range(NIB):
            out_sb = outp.tile([128, H * N_C], OUT_DT)
            stats = stat_pool.tile([128, H], F32, tag="sums")
            recs = stat_pool.tile([128, H], F32, tag="recs")
            for h in range(H):
                q, hh = h // 2, h % 2
                lhsT = a2T[hh * DH : (hh + 1) * DH, q * N_C + ib * 128 : q * N_C + (ib + 1) * 128]
                rhs_all = xT[hh * DH : (hh + 1) * DH, q * N_C : (q + 1) * N_C]
                psc = spsum.tile([128, N_C], F32)
                for jc in range(2):
                    nc.tensor.matmul(
                        psc[:, jc * 512 : (jc + 1) * 512],
                        lhsT,
                        rhs_all[:, jc * 512 : (jc + 1) * 512],
                        start=True,
                        stop=True,
                    )
                exp_slice = out_sb[:, h * N_C : (h + 1) * N_C]
                nc.scalar.activation(
                    exp_slice,
                    psc[:],
                    mybir.ActivationFunctionType.Exp,
                    accum_out=stats[:, h : h + 1],
                )
                nc.vector.reciprocal(recs[:, h : h + 1], stats[:, h : h + 1])
                nc.vector.tensor_scalar_mul(exp_slice, exp_slice, recs[:, h : h + 1])
            nc.sync.dma_start(
                out_ap[s, ib * 128 : (ib + 1) * 128, :, :],
                out_sb[:].rearrange("p (h j) -> p h j", h=H),
            )


def _split_multi_waits(nc):
    """walrus's per-instruction codegen structs hold only one embedded sync
    wait; hoist multi-wait instructions' waits onto standalone same-engine
    wait instructions placed immediately before them (program order on the
    sequencer preserves semantics)."""
    ctr = 0
    for f in nc.m.functions:
        for blk in f.blocks:
            out = []
            changed = False
            for inst in blk.instructions:
                tname = type(inst).__name__
                si = inst.sync_info
                if (
                    tname != "InstEventSemaphore"
                    and si is not None
                    and si.on_wait
                    and len(si.on_wait) > 1
                ):
                    for w in si.on_wait:
                        wi = mybir.InstEventSemaphore(name=f"WSPLIT-{ctr}")
                        ctr += 1
                        wi.engine = inst.engine
                        wi.sync_info = mybir.SyncInfo(on_wait=[w], on_update=[])
                        out.append(wi)
                    inst.sync_info = mybir.SyncInfo(
                        on_wait=[], on_update=list(si.on_update)
                    )
                    changed = True
                out.append(inst)
            if changed:
                blk.instructions = out
    return ctr


def build_bass(bench_repeats=None, split_waits=True):
    nc = bass.Bass("TRN2", target_bir_lowering=False, debug=False)
    if bench_repeats is None:
        x_ap = nc.dram_tensor(
            "x", [SLABS_PER_CORE, N_C, D], F32, kind="ExternalInput"
        ).ap()
        w_ap = nc.dram_tensor(
            "attn_w", [SLABS_PER_CORE, H, N_C, 2 * DH], F32, kind="ExternalInput"
        ).ap()
        out_ap = nc.dram_tensor(
            "out", [SLABS_PER_CORE, N_C, H, N_C], OUT_DT, kind="ExternalOutput"
        ).ap()
        with tile.TileContext(nc) as tc:
            with ExitStack() as ctx:
                pools = make_pools(ctx, tc)
                build_kernel_body(pools, tc, out_ap, x_ap, w_ap)
    else:
        # bench variant: all big tensors are device-internal (no host I/O);
        # tiny external in/out keep the custom-call ABI happy. Internal
        # inputs are zeroed once, then the body runs `bench_repeats` times
        # (unrolled; For_i trips a walrus InstISA codegen bug).
        x_ap = nc.dram_tensor("xi", [SLABS_PER_CORE, N_C, D], F32).ap()
        w_ap = nc.dram_tensor("wi", [SLABS_PER_CORE, H, N_C, 2 * DH], F32).ap()
        out_ap = nc.dram_tensor("oi", [SLABS_PER_CORE, N_C, H, N_C], OUT_DT).ap()
        tin = nc.dram_tensor("tin", [1, 4], F32, kind="ExternalInput").ap()
        tout = nc.dram_tensor("tout", [1, 4], F32, kind="ExternalOutput").ap()
        with tile.TileContext(nc) as tc:
            with ExitStack() as ctx:
                pools = make_pools(ctx, tc)
                tiny = pools["const"].tile([1, 4], F32)
                nc.gpsimd.dma_start(tiny[:], tin[:, :])
                nc.gpsimd.dma_start(tout[:, :], tiny[:])
                zt = pools["const"].tile([128, 4 * N_C], F32)
                nc.vector.memset(zt[:], 0.0)
                x_flat = x_ap.rearrange("s (a p) d -> (s a) p d", p=128)
                for t in range(x_flat.shape[0]):
                    nc.gpsimd.dma_start(x_flat[t], zt[:, :D])
                w_flat = w_ap.rearrange("s h (a p) k -> (s h a) p k", p=128)
                for t in range(w_flat.shape[0]):
                    nc.gpsimd.dma_start(w_flat[t], zt[:, : 2 * DH])
                for _ in range(bench_repeats):
                    build_kernel_body(pools, tc, out_ap, x_ap, w_ap)
    if split_waits:
        _split_multi_waits(nc)
    return nc


_NC_CACHE = None


def _get_nc():
    global _NC_CACHE
    if _NC_CACHE is None:
        _NC_CACHE = build_bass()
    return _NC_CACHE


def kernel(x: np.ndarray, attn_w: np.ndarray, _trace: bool = False):
    assert x.shape == (4, 4, N_C, D), x.shape
    assert attn_w.shape == (4, 4, H, N_C, 2 * DH), attn_w.shape
    xs = np.ascontiguousarray(x, dtype=np.float32).reshape(16, N_C, D)
    ws = np.ascontiguousarray(attn_w, dtype=np.float32).reshape(16, H, N_C, 2 * DH)
    in_maps = [
        {
            "x": np.ascontiguousarray(xs[2 * c : 2 * c + 2]),
            "attn_w": np.ascontiguousarray(ws[2 * c : 2 * c + 2]),
        }
        for c in range(NUM_CORES)
    ]
    nc = _get_nc()
    res = run_bass_kernel_spmd(
        nc, in_maps, core_ids=list(range(NUM_CORES)), trace=_trace
    )
    out = np.concatenate([res.results[c]["out"] for c in range(NUM_CORES)], axis=0)
    if _trace:
        kernel.last_exec_time_ns = res.exec_time_ns
        it = res.instructions_and_trace
        kernel.last_trace_path = it[1] if it else None
    return out.reshape(4, 4, N_C, H, N_C).astype(np.float32)


kernel.last_exec_time_ns = None
kernel.last_trace_path = None
